# revision 20
# baseline (speedup 1.0000x reference)
"""Trainium2 Bass kernel for nn_AdaptiveValuesMetadataAttention.

Shapes (hardcoded from the problem spec):
  values   [1, 8, 512, 256]  metadata [1, 8, 512, 64]
  w_meta_outer [64, 512]  w_qkv [256, 768]  w_meta_inner [64, 512]
  w_out [256, 256]  b_out [256]

Strategy: the outer source-level metadata attention selects, per source s,
the top-3 source windows (the +2*I diagonal boost guarantees slot 0 == s).
That top-k and the window gather are data-dependent *sharding* and run on
the host.  Each of the 8 NeuronCores then computes one source's inner
fused attention (queries = window slot-0 tokens, keys/values = all 3*512
window tokens) entirely on-device.

Per-core device kernel (matmul operands bf16, fp32 psum accumulate).
Heads are processed in pairs (2t, 2t+1) with pair-interleaved layouts so
every hot matmul is one full-width PE instruction:

  phase 1: combined-contraction projections.  The host packs zero-padded
    weights over the stacked [values(256); metadata(64)] contraction so a
    single psum tile accumulates the pair-interleaved
    [Qp_a|Qm_a|Qp_b|Qm_b] / [Kp_a|Km_a|Kp_b|Km_b] rows (clip fused into
    the psum->sbuf copy).  V is emitted token-major into a pair-packed
    [Va|1|0...|Vb|1|0...] column layout (the ones-column yields softmax
    denominators; the zero columns make the attn@V output land on
    32-aligned partition bands).
  phase 2: per pair: scoresT for both heads via K=64 matmuls on disjoint
    64-row PE groups; exp on the scalar engine (fused 1/sqrt(dh) scale);
    attn@V for both heads in one M=128 matmul per chunk.  Denominators
    are broadcast across partitions with K=1 matmuls and inverted with
    the fast DVE reciprocal.
  phase 3: output projection (pair-packed K=128 accumulation) + bias,
    emitted transposed [dv, n]; the host transposes and stacks cores.
"""

import numpy as np

B, S, N, DV, DM = 1, 8, 512, 256, 64
INNER, H, WS = 256, 8, 3
DH = INNER // H          # 32
W = WS * N               # 1536 kv tokens per window
SCALE = DH ** -0.5

_CACHE = {}


def _host_top_idx(values, metadata, w_meta_outer):
    meta_mean = metadata.mean(axis=2)                        # [B,S,DM]
    qk = meta_mean @ w_meta_outer                            # [B,S,2*INNER]
    qm = np.clip(qk[..., :INNER], -5, 5)
    km = np.clip(qk[..., INNER:], -5, 5)
    dots = np.einsum('bqd,bkd->bqk', qm, km) * (INNER ** -0.5)
    m = dots.max(-1, keepdims=True)
    e = np.exp(dots - m)
    attn = e / e.sum(-1, keepdims=True)
    attn = attn + 2.0 * np.eye(S, dtype=attn.dtype)
    # jax.lax.top_k: k largest, ties broken by lower index (stable)
    return np.argsort(-attn, axis=-1, kind='stable')[..., :WS]  # [B,S,WS]


def _build_bass():
    import concourse.bass as bass  # noqa: F401
    import concourse.tile as tile
    from concourse import bacc, mybir

    F32 = mybir.dt.float32
    BF16 = mybir.dt.bfloat16
    EXP = mybir.ActivationFunctionType.Exp
    MIN = mybir.AluOpType.min
    MAX = mybir.AluOpType.max

    nc = bacc.Bacc(None, target_bir_lowering=False)

    kvT = nc.dram_tensor("kvT", [DV, W], BF16, kind="ExternalInput")
    kvmT = nc.dram_tensor("kvmT", [DM, W], BF16, kind="ExternalInput")
    # combined-contraction projection weights, pair-interleaved columns:
    # wc12 = [k-part1 | k-part2 | q-part1 | q-part2]  (kvT rows 0:128/128:256)
    # wc3  = [k-part3 | q-part3]                      (kvmT rows)
    wc12 = nc.dram_tensor("wc12", [128, 2048], BF16, kind="ExternalInput")
    wc3 = nc.dram_tensor("wc3", [DM, 1024], BF16, kind="ExternalInput")
    wv = nc.dram_tensor("wv", [DV, INNER], BF16, kind="ExternalInput")
    # output weights packed per head: col block h = wo[32h:32h+32, :]
    woh = nc.dram_tensor("woh", [32, H * DV], BF16, kind="ExternalInput")
    bo = nc.dram_tensor("bo", [128, 2], F32, kind="ExternalInput")
    out = nc.dram_tensor("out", [DV, N], F32, kind="ExternalOutput")

    with tile.TileContext(nc) as tc:
        with (
            tc.tile_pool(name="w", bufs=1) as wp,
            tc.tile_pool(name="big", bufs=1) as bigp,
            tc.tile_pool(name="expp", bufs=6) as expp,
            tc.tile_pool(name="tails", bufs=3) as tailsb,
        ):
            # ---- persistent SBUF: inputs + weights --------------------
            kvT_sb = [wp.tile([128, W], BF16, tag=f"kvT{d}", name=f"kvT{d}")
                      for d in range(2)]
            nc.sync.dma_start(out=kvT_sb[0][:], in_=kvT[0:128, :])
            nc.gpsimd.dma_start(out=kvT_sb[1][:], in_=kvT[128:256, :])
            kvmT_sb = wp.tile([DM, W], BF16, tag="kvmT")
            nc.sync.dma_start(out=kvmT_sb[:], in_=kvmT[:])
            wc12_sb = wp.tile([128, 2048], BF16, tag="wc12")
            nc.sync.dma_start(out=wc12_sb[:], in_=wc12[:])
            wc3_sb = wp.tile([DM, 1024], BF16, tag="wc3")
            nc.gpsimd.dma_start(out=wc3_sb[:], in_=wc3[:])
            wv_sb = wp.tile([128, 2 * INNER], BF16, tag="wv")
            nc.gpsimd.dma_start(out=wv_sb[:, 0:INNER], in_=wv[0:128, :])
            nc.gpsimd.dma_start(out=wv_sb[:, INNER:], in_=wv[128:256, :])
            woh_sb = wp.tile([32, H * DV], BF16, tag="woh")
            nc.sync.dma_start(out=woh_sb[:], in_=woh[:])
            wo_sb = [woh_sb[:, DV * h:DV * (h + 1)] for h in range(H)]
            b_sb = wp.tile([128, 2], F32, tag="b")
            nc.sync.dma_start(out=b_sb[:], in_=bo[:])
            ones_sb = wp.tile([64, 32], BF16, tag="ones")
            nc.vector.memset(ones_sb[:], 1.0)

            # ---- persistent SBUF: projection outputs ------------------
            # QcatT/KcatT tile t: rows [Qp_2t|Qm_2t|Qp_2t+1|Qm_2t+1] etc.
            # Qz[t][0] = [Qcat_a rows; 0], Qz[t][1] = [0; Qcat_b rows] so the
            # score matmuls use full K=128 weights (enables FWL).
            Qz_sb = [[bigp.tile([128, N], BF16, tag=f"Qz{t}{u}",
                                name=f"Qz{t}{u}") for u in range(2)]
                     for t in range(4)]
            for t in range(4):
                nc.vector.memset(Qz_sb[t][0][64:128, :], 0.0)
                nc.vector.memset(Qz_sb[t][1][0:64, :], 0.0)
            KcatT_sb = [bigp.tile([128, W], BF16, tag=f"Kc{t}", name=f"Kc{t}")
                        for t in range(4)]
            # V chunk c: per pair t a 128-col block [Va|1|0*31|Vb|1|0*31]
            V_sb = [bigp.tile([128, 512], BF16, tag=f"V{c}", name=f"V{c}")
                    for c in range(12)]
            OTn_sb = [bigp.tile([32, N], BF16, tag=f"OTn{h}", name=f"OTn{h}")
                      for h in range(H)]

            def clip_copy(dst, src):
                nc.vector.tensor_scalar(dst, src, 5.0, -5.0, MIN, MAX)

            # ---- phase 0: PE warm-up during the input DMA wait --------
            # ~56 small matmuls on constant data keep the PE busy so the
            # HAM clock gate reaches K=8/8 before the real work arrives.
            with tc.tile_pool(name="warm", bufs=1, space="PSUM") as warmp:
                wps = warmp.tile([32, 32], F32, tag="warm", name="wps")
                for i in range(290):
                    nc.tensor.matmul(wps[:], ones_sb[0:64, :],
                                     ones_sb[0:64, :])

            # ---- phase 1: projections ---------------------------------
            with tc.tile_pool(name="proj", bufs=6, space="PSUM") as projp:
                for t in range(4):
                    cs = slice(128 * t, 128 * (t + 1))
                    ps = projp.tile([128, N], F32, tag="proj", name="psq")
                    nc.tensor.matmul(ps[:], wc12_sb[:, 1024:1536][:, cs],
                                     kvT_sb[0][:, 0:N], start=True, stop=False)
                    nc.tensor.matmul(ps[:], wc12_sb[:, 1536:2048][:, cs],
                                     kvT_sb[1][:, 0:N], start=False, stop=False)
                    nc.tensor.matmul(ps[:], wc3_sb[:, 512:1024][:, cs],
                                     kvmT_sb[:, 0:N], start=False, stop=True)
                    clip_copy(Qz_sb[t][0][0:64, :], ps[0:64, :])
                    clip_copy(Qz_sb[t][1][64:128, :], ps[64:128, :])
                for bk in range(3):
                    fs = slice(512 * bk, 512 * (bk + 1))
                    for t in range(4):
                        cs = slice(128 * t, 128 * (t + 1))
                        ps = projp.tile([128, N], F32, tag="proj", name="psk")
                        nc.tensor.matmul(ps[:], wc12_sb[:, 0:512][:, cs],
                                         kvT_sb[0][:, fs], start=True, stop=False)
                        nc.tensor.matmul(ps[:], wc12_sb[:, 512:1024][:, cs],
                                         kvT_sb[1][:, fs], start=False, stop=False)
                        nc.tensor.matmul(ps[:], wc3_sb[:, 0:512][:, cs],
                                         kvmT_sb[:, fs], start=False, stop=True)
                        clip_copy(KcatT_sb[t][:, fs], ps[:])
                for c in range(12):
                    cs = slice(128 * c, 128 * (c + 1))
                    ps = projp.tile([128, DV], F32, tag="proj", name="psv")
                    nc.tensor.matmul(ps[:], kvT_sb[0][:, cs], wv_sb[:, 0:INNER],
                                     start=True, stop=False)
                    nc.tensor.matmul(ps[:], kvT_sb[1][:, cs], wv_sb[:, INNER:],
                                     start=False, stop=True)
                    # pair-packed V: even heads -> col 128u, odd -> 128u+64
                    s4 = ps[:].rearrange("p (u x w) -> p u x w", u=4, w=32)
                    d4 = V_sb[c][:].rearrange("p (u y w) -> p u y w", u=4, w=32)
                    nc.vector.tensor_copy(d4[:, :, 0, :], s4[:, :, 0, :])
                    nc.vector.tensor_copy(d4[:, :, 2, :], s4[:, :, 1, :])
                    v64 = V_sb[c][:].rearrange("p (v w) -> p v w", w=64)
                    nc.vector.memset(v64[:, :, 32:33], 1.0)
                    nc.vector.memset(v64[:, :, 33:64], 0.0)

            # ---- phase 2: attention per head pair ---------------------
            NBLK = 4
            with (
                tc.tile_pool(name="sc", bufs=2, space="PSUM") as scp,
                tc.tile_pool(name="tail", bufs=2, space="PSUM") as tailp,
            ):
                sumsp = tailp
                for t in range(4):
                    outpsA = tailp.tile([33, N], F32, tag="outps", name="outpsA")
                    outpsB = tailp.tile([33, N], F32, tag="outps", name="outpsB")
                    for blk in range(NBLK):
                        psA = scp.tile([128, 1536], F32, tag="sc", name="psA")
                        psB = scp.tile([128, 1536], F32, tag="sc", name="psB")
                        for j in range(3):
                            c = 3 * blk + j
                            cs = slice(128 * c, 128 * (c + 1))
                            js = slice(512 * j, 512 * (j + 1))
                            nc.tensor.matmul(
                                psA[:, js], KcatT_sb[t][:, cs],
                                Qz_sb[t][0][:])
                            nc.tensor.matmul(
                                psB[:, js], KcatT_sb[t][:, cs],
                                Qz_sb[t][1][:])
                        eA = expp.tile([128, 1536], BF16, tag="exp", name="eA")
                        eB = expp.tile([128, 1536], BF16, tag="exp", name="eB")
                        nc.scalar.activation(eA[:], psA[:], EXP, scale=SCALE)
                        nc.scalar.activation(eB[:], psB[:], EXP, scale=SCALE)
                        for j in range(3):
                            c = 3 * blk + j
                            js = slice(512 * j, 512 * (j + 1))
                            nc.tensor.matmul(
                                outpsA[0:33, :],
                                V_sb[c][:, 128 * t:128 * t + 33],
                                eA[:, js], start=(c == 0), stop=(c == 11))
                            nc.tensor.matmul(
                                outpsB[0:33, :],
                                V_sb[c][:, 128 * t + 64:128 * t + 97],
                                eB[:, js], start=(c == 0), stop=(c == 11))
                    # tail: normalize each head of the pair
                    for h, outps in ((2 * t, outpsA), (2 * t + 1, outpsB)):
                        stg = tailsb.tile([64, N], BF16, tag="stg", name="stg")
                        nc.vector.tensor_copy(stg[0:33, :], outps[0:33, :])
                        sums_ps = sumsp.tile([32, N], F32, tag="outps",
                                             name="sums_ps")
                        nc.tensor.matmul(sums_ps[:], ones_sb[32:33, 0:32],
                                         stg[32:33, :], tile_position=(32, 0))
                        rcp = tailsb.tile([32, N], F32, tag="rcp", name="rcp")
                        nc.vector.reciprocal_approx_fast(out=rcp[:],
                                                         in_=sums_ps[:])
                        nc.vector.tensor_mul(OTn_sb[h][:], stg[0:32, :], rcp[:])

            # ---- phase 3: output projection + bias --------------------
            with tc.tile_pool(name="fin", bufs=2, space="PSUM") as finp:
                for d in range(2):
                    sl = slice(128 * d, 128 * (d + 1))
                    ops = finp.tile([128, N], F32, tag="fin", name="ops")
                    for h in range(H):
                        nc.tensor.matmul(ops[:], wo_sb[h][:, sl], OTn_sb[h][:],
                                         start=(h == 0), stop=(h == H - 1))
                    fin = tailsb.tile([128, N], F32, tag="fin", name="fin")
                    nc.vector.tensor_scalar_add(fin[:], ops[:], b_sb[:, d:d + 1])
                    nc.sync.dma_start(out=out[sl, :], in_=fin[:])

    nc.compile()
    return nc


def _get_nc():
    if "nc" not in _CACHE:
        _CACHE["nc"] = _build_bass()
    return _CACHE["nc"]


def _pack_weights(w_qkv, w_meta_inner, w_out, b_out):
    import ml_dtypes
    bf = ml_dtypes.bfloat16
    f = np.float32
    wq = w_qkv[:, :INNER]
    wk = w_qkv[:, INNER:2 * INNER]
    wv = w_qkv[:, 2 * INNER:]
    wmq = w_meta_inner[:, :INNER]
    wmk = w_meta_inner[:, INNER:]

    # combined-contraction pair-interleaved projection weights
    def cat_pack(wp_, wm_):
        # returns part1 [128,512], part2 [128,512], part3 [64,512]
        p1 = np.zeros((128, 512), dtype=np.float32)
        p2 = np.zeros((128, 512), dtype=np.float32)
        p3 = np.zeros((64, 512), dtype=np.float32)
        for t in range(4):
            a, b2 = 2 * t, 2 * t + 1
            c0 = 128 * t
            p1[:, c0 + 0:c0 + 32] = wp_[0:128, 32 * a:32 * a + 32]
            p2[:, c0 + 0:c0 + 32] = wp_[128:256, 32 * a:32 * a + 32]
            p3[:, c0 + 32:c0 + 64] = wm_[:, 32 * a:32 * a + 32]
            p1[:, c0 + 64:c0 + 96] = wp_[0:128, 32 * b2:32 * b2 + 32]
            p2[:, c0 + 64:c0 + 96] = wp_[128:256, 32 * b2:32 * b2 + 32]
            p3[:, c0 + 96:c0 + 128] = wm_[:, 32 * b2:32 * b2 + 32]
        return p1, p2, p3

    k1, k2, k3 = cat_pack(wk, wmk)
    q1, q2, q3 = cat_pack(wq, wmq)
    wc12 = np.ascontiguousarray(
        np.concatenate([k1, k2, q1, q2], axis=1)).astype(bf)   # [128, 2048]
    wc3 = np.ascontiguousarray(
        np.concatenate([k3, q3], axis=1)).astype(bf)           # [64, 1024]

    woh = np.ascontiguousarray(np.concatenate(
        [w_out[32 * h:32 * h + 32, :] for h in range(H)], axis=1)).astype(bf)

    bo = np.ascontiguousarray(
        np.stack([b_out[0:128], b_out[128:256]], axis=1), dtype=f)
    wv_bf = np.ascontiguousarray(wv).astype(bf)
    return {"wc12": wc12, "wc3": wc3, "wv": wv_bf, "woh": woh, "bo": bo}


def build_in_maps(values, metadata, w_qkv, w_meta_inner, w_out, b_out, top_idx):
    import ml_dtypes
    bf = ml_dtypes.bfloat16
    shared = _pack_weights(w_qkv, w_meta_inner, w_out, b_out)
    in_maps = []
    for s in range(S):
        idx = top_idx[0, s]
        kvT = np.ascontiguousarray(values[0, idx].reshape(W, DV).T).astype(bf)
        kvmT = np.ascontiguousarray(metadata[0, idx].reshape(W, DM).T).astype(bf)
        in_maps.append({"kvT": kvT, "kvmT": kvmT, **shared})
    return in_maps


def kernel(values, metadata, w_meta_outer, w_qkv, w_meta_inner, w_out, b_out,
           _trace=False):
    from concourse.bass_utils import run_bass_kernel_spmd

    values = np.asarray(values, dtype=np.float32)
    metadata = np.asarray(metadata, dtype=np.float32)
    w_meta_outer = np.asarray(w_meta_outer, dtype=np.float32)
    w_qkv = np.asarray(w_qkv, dtype=np.float32)
    w_meta_inner = np.asarray(w_meta_inner, dtype=np.float32)
    w_out = np.asarray(w_out, dtype=np.float32)
    b_out = np.asarray(b_out, dtype=np.float32)

    top_idx = _host_top_idx(values, metadata, w_meta_outer)
    assert (top_idx[0, :, 0] == np.arange(S)).all(), top_idx

    in_maps = build_in_maps(values, metadata, w_qkv, w_meta_inner, w_out,
                            b_out, top_idx)
    nc = _get_nc()
    res = run_bass_kernel_spmd(nc, in_maps, core_ids=list(range(S)),
                               trace=_trace)
    out = np.stack([res.results[s]["out"].T for s in range(S)], axis=0)
    _CACHE["last_result"] = res
    return out.reshape(B, S, N, DV)
